# revision 1
# baseline (speedup 1.0000x reference)
"""Bi-directional minGRU kernel for Trainium2 (8 NeuronCores, Bass/Tile).

Strategy
--------
Data-parallel over batch: B=256 examples sharded 32 per core. Per example all
tensors live in feature-major layout [feature->partition, time->free], so every
linear layer is a TensorE matmul with K=features on partitions, and the minGRU
recurrence is a single hardware `tensor_tensor_scan` along the free (time)
axis: rows 0-63 carry the forward direction in normal time order, rows 64-127
carry the backward direction in *reversed* time order (the reversal is free:
all backward-direction matmul rhs operands are negative-stride views).

Host-side (numpy, fp64 then cast to fp32) the linear chains are fused:
    gz_d = (wz@proj[:, :3]) @ x3 + (wz@proj[:, 3:]@te_w2) @ relu_te1 + bias
so the time encoder's second layer, the input projections, and the gate
weights collapse into single [67 -> 64] matmuls, and the head's te branch
collapses into W1t2 = gh_w1[:,128:] @ te_w2.

The masked-position fixup  h_apply = m*pre + (1-m)*final  commutes with the
head matmul (m is a per-time scalar):  W@h_apply = W@(m*(pre-final)) + W@final,
so the kernel computes E = m*(pre - final) elementwise, one extra [128,1]
matmul for W@final (folded into the head bias), and never materializes
h_apply.
"""
import os
import sys

for _p in ("/opt/trn_rl_repo", "/root/.axon_site/_ro/trn_rl_repo"):
    if os.path.isdir(_p) and _p not in sys.path:
        sys.path.insert(0, _p)

import numpy as np
from contextlib import ExitStack

import concourse.bacc as bacc
import concourse.tile as tile
import concourse.mybir as mybir
from concourse.bass_utils import run_bass_kernel_spmd

F32 = mybir.dt.float32
AF = mybir.ActivationFunctionType
OP = mybir.AluOpType

B, L, H, TE = 256, 2048, 64, 64
NCORES = 8
BS = B // NCORES          # examples per core
NW = 709                  # packed weight columns (see _pack_weights)

# weight column layout inside the packed [128, NW] tile
_C_W1FB = 0               # lhsT of [W1f | W1b], [128, 128]
_C_W1T2 = 128             # lhsT of W1t2, rows 0-63, [64, 128]
_C_ZF = 256               # gate lhsTs, rows 0-66, [67, 64] each
_C_HF = 320
_C_ZB = 384
_C_HB = 448
_C_A1 = 512               # te_w1 lhsT, row 96 only, [1, 64]
_C_W2T = 576              # gh_w2^T, [128, 1]
_C_ZBIAS = 577            # gate z bias column [128, 1]
_C_HBIAS = 578            # gate h bias column
_C_HEADB = 579            # head bias column (gh_b1 + W1t@te_b2)
_C_B1 = 580               # te bias, rows 0-63
_C_W1B0 = 581             # lhsT of W1b zero-padded to K=128 (rows 0-63 zero)

_cache = {}


def _pack_weights(inp):
    """Fuse the linear chains (fp64) and pack every lhsT into one [128, NW]
    fp32 array whose column slices are the matmul stationary operands."""
    g = {k: np.asarray(v, np.float64) for k, v in inp.items()}
    wts = np.zeros((128, NW), np.float64)

    def fuse(proj_w, proj_b, wz, bz, wh, bh):
        P3 = proj_w[:, :3]
        Pte_te2 = proj_w[:, 3:] @ g["te_w2"]
        pbias = proj_w[:, 3:] @ g["te_b2"] + proj_b
        # x3 rows on the device are ordered [mask, x1, x2]
        reord = np.stack([P3[:, 2], P3[:, 0], P3[:, 1]], axis=1)
        return (
            np.concatenate([wz @ Pte_te2, wz @ reord], axis=1),  # (64, 67)
            wz @ pbias + bz,
            np.concatenate([wh @ Pte_te2, wh @ reord], axis=1),
            wh @ pbias + bh,
        )

    Zf, zbf, Hf, hbf = fuse(g["fproj_w"], g["fproj_b"], g["fwz"], g["fbz"],
                            g["fwh"], g["fbh"])
    Zb, zbb, Hb, hbb = fuse(g["bproj_w"], g["bproj_b"], g["bwz"], g["bbz"],
                            g["bwh"], g["bbh"])
    # gate lhsT: [K=67 rows: 0-63 r, 64 mask, 65 x1, 66 x2][M=64]
    wts[0:67, _C_ZF:_C_ZF + 64] = Zf.T
    wts[0:67, _C_HF:_C_HF + 64] = Hf.T
    wts[0:67, _C_ZB:_C_ZB + 64] = Zb.T
    wts[0:67, _C_HB:_C_HB + 64] = Hb.T
    wts[0:64, _C_ZBIAS] = zbf
    wts[64:128, _C_ZBIAS] = zbb
    wts[0:64, _C_HBIAS] = hbf
    wts[64:128, _C_HBIAS] = hbb
    # head
    W1f = g["gh_w1"][:, :64]
    W1b = g["gh_w1"][:, 64:128]
    W1t = g["gh_w1"][:, 128:192]
    W1fb = np.concatenate([W1f, W1b], axis=1)          # (128, 128)
    wts[0:128, _C_W1FB:_C_W1FB + 128] = W1fb.T
    wts[0:64, _C_W1T2:_C_W1T2 + 128] = (W1t @ g["te_w2"]).T
    wts[0:128, _C_HEADB] = g["gh_b1"] + W1t @ g["te_b2"]
    # te first layer: lhsT row 96 (rhs = tn row at partition 96)
    wts[96, _C_A1:_C_A1 + 64] = g["te_w1"][:, 0]
    wts[0:64, _C_B1] = g["te_b1"]
    # head2
    wts[0:128, _C_W2T] = g["gh_w2"][0]
    # U-b lhsT: rows 64-127 carry W1b^T, rows 0-63 stay zero so the matmul
    # can run as a full K=128 (legal PE tiling); the top rows multiply the
    # reversed forward-half of E and are zeroed out.
    wts[64:128, _C_W1B0:_C_W1B0 + 128] = W1b.T
    return np.ascontiguousarray(wts, np.float32), np.float32(g["gh_b2"][0])


def _pack_inputs(x, t):
    """Per-example input rows [mask, x1_sub, x2_sub, t] -> (B, 4, L) fp32."""
    x = np.asarray(x, np.float32)
    t = np.asarray(t, np.float32)
    xT = np.swapaxes(x, 1, 2)                  # (B, 3, L)
    mask = xT[:, 2:3, :]
    return mask, xT, t


def _build_program():
    """Build + compile the 8-core SPMD Bass program once."""
    nc = bacc.Bacc("TRN2", num_devices=NCORES, debug=False)
    wts_d = nc.dram_tensor("wts", [128, NW], F32, kind="ExternalInput")
    inx_d = nc.dram_tensor("inx", [BS, 4, L], F32, kind="ExternalInput")
    out_d = nc.dram_tensor("out", [BS, L], F32, kind="ExternalOutput")

    with tile.TileContext(nc) as tc, ExitStack() as ctx:
        wpool = ctx.enter_context(tc.tile_pool(name="w", bufs=1))
        pool = ctx.enter_context(tc.tile_pool(name="p", bufs=2))
        spool = ctx.enter_context(tc.tile_pool(name="s", bufs=2))
        ps_half = ctx.enter_context(tc.tile_pool(name="ph", bufs=4, space="PSUM"))
        ps_one = ctx.enter_context(tc.tile_pool(name="p1", bufs=1, space="PSUM"))

        wts = wpool.tile([128, NW], F32, tag="wts")
        nc.sync.dma_start(wts[:], wts_d.ap()[:])
        inx = inx_d.ap()

        psP = None  # per-4-example-group preds psum tiles (one per L-chunk)
        for e in range(BS):
            eg = e % 4
            # ---- input staging -------------------------------------------
            xr = pool.tile([128, L], F32, tag="xr")      # 0-63 r, 64-66 x3, 96 t
            nc.sync.dma_start(xr[64:67, :], inx[e, 0:3, :])
            nc.sync.dma_start(xr[96:97, :], inx[e, 3:4, :])
            m128 = pool.tile([128, L], F32, tag="m128")  # mask bcast; rows
            nc.sync.dma_start(m128[0:64, :],             # 64-127 reversed
                              inx[e, 0:1, :].broadcast_to((64, L)))
            nc.sync.dma_start(m128[64:128, :],
                              inx[e, 0:1, ::-1].broadcast_to((64, L)))
            xrev = xr[0:67, ::-1]

            # ---- time encoder r = relu(A1 @ t + b1) ----------------------
            for c in range(4):
                cs = slice(c * 512, (c + 1) * 512)
                pst = ps_half.tile([128, 512], F32, tag="ph")
                nc.tensor.matmul(pst[0:64, :],
                                 wts[96:97, _C_A1:_C_A1 + 64],
                                 xr[96:97, cs], start=True, stop=True,
                                 tile_position=(96, 0))
                nc.scalar.activation(xr[0:64, cs], pst[0:64, :], AF.Relu,
                                     bias=wts[0:64, _C_B1:_C_B1 + 1])

            # ---- gates ----------------------------------------------------
            Z = pool.tile([128, L], F32, tag="Z")
            TH = pool.tile([128, L], F32, tag="TH")
            for (dst, cf, cb, bias_c, fn) in (
                    (Z, _C_ZF, _C_ZB, _C_ZBIAS, AF.Sigmoid),
                    (TH, _C_HF, _C_HB, _C_HBIAS, AF.Tanh)):
                for c in range(4):
                    cs = slice(c * 512, (c + 1) * 512)
                    psg = ps_half.tile([128, 512], F32, tag="ph")
                    nc.tensor.matmul(psg[0:64, :],
                                     wts[0:67, cf:cf + 64],
                                     xr[0:67, cs], start=True, stop=True,
                                     tile_position=(0, 0))
                    nc.tensor.matmul(psg[64:128, :],
                                     wts[0:67, cb:cb + 64],
                                     xrev[:, cs], start=True, stop=True,
                                     tile_position=(0, 64))
                    nc.scalar.activation(dst[:, cs], psg[:], fn,
                                         bias=wts[:, bias_c:bias_c + 1])

            # ---- scan inputs: a = 1-z (DVE), b = z*th (GPSIMD) -----------
            A = pool.tile([128, L], F32, tag="A")
            Bt = pool.tile([128, L], F32, tag="Bt")
            nc.vector.tensor_scalar(A[:], Z[:], -1.0, 1.0, OP.mult, OP.add)
            for hh in range(2):
                hs = slice(hh * 1024, (hh + 1) * 1024)
                nc.gpsimd.tensor_tensor(Bt[:, hs], Z[:, hs], TH[:, hs],
                                        OP.mult)

            # ---- the scan -------------------------------------------------
            Hs = pool.tile([128, L + 1], F32, tag="Hs")
            nc.vector.memset(Hs[:, 0:1], 0.0)
            nc.vector.tensor_tensor_scan(Hs[:, 1:L + 1], A[:], Bt[:], 0.0,
                                         OP.mult, OP.add)

            # ---- masked fixup E = m * (pre - final) ----------------------
            Dt = pool.tile([128, L], F32, tag="Dt")
            nc.vector.tensor_scalar(Dt[:], Hs[:, 0:L],
                                    Hs[:, L - 1:L], None, OP.subtract)
            Et = pool.tile([128, L], F32, tag="Et")
            nc.gpsimd.tensor_tensor(Et[:], Dt[:], m128[:], OP.mult)
            Erev = Et[0:128, ::-1]

            # ---- head bias: W1fb @ final + headb -------------------------
            psv = ps_half.tile([128, 512], F32, tag="ph")
            nc.tensor.matmul(psv[:, 0:1], wts[:, _C_W1FB:_C_W1FB + 128],
                             Hs[:, L - 1:L], start=True, stop=True,
                             tile_position=(0, 0))
            sbb = spool.tile([128, 4], F32, tag="sbb")
            nc.scalar.activation(sbb[:, 0:1], psv[:, 0:1], AF.Identity,
                                 bias=wts[:, _C_HEADB:_C_HEADB + 1])

            # ---- head layer 1 --------------------------------------------
            hid = pool.tile([128, L], F32, tag="hid")
            for c in range(4):
                cs = slice(c * 512, (c + 1) * 512)
                psS = ps_half.tile([128, 512], F32, tag="ph")
                nc.tensor.matmul(psS[:],
                                 wts[0:64, _C_W1FB:_C_W1FB + 128],
                                 Et[0:64, cs], start=True, stop=False,
                                 tile_position=(0, 0))
                nc.tensor.matmul(psS[:],
                                 wts[0:128, _C_W1B0:_C_W1B0 + 128],
                                 Erev[:, cs], start=False, stop=False,
                                 tile_position=(0, 0))
                nc.tensor.matmul(psS[:],
                                 wts[0:64, _C_W1T2:_C_W1T2 + 128],
                                 xr[0:64, cs], start=False, stop=True,
                                 tile_position=(0, 0))
                nc.scalar.activation(hid[:, cs], psS[:], AF.Relu,
                                     bias=sbb[:, 0:1])

            # ---- head layer 2: preds for 4 examples share one psum tile
            # per L-chunk, at partition rows 0/32/64/96 ---------------------
            if eg == 0:
                psP = [ps_one.tile([128, 512], F32, tag=f"p1_{c}",
                                   name=f"psP{e}_{c}")
                       for c in range(4)]
            for c in range(4):
                cs = slice(c * 512, (c + 1) * 512)
                nc.tensor.matmul(psP[c][32 * eg:32 * eg + 1, :],
                                 wts[:, _C_W2T:_C_W2T + 1],
                                 hid[:, cs], start=True, stop=True,
                                 tile_position=(0, 32 * eg))
            if eg == 3:
                for c in range(4):
                    cs = slice(c * 512, (c + 1) * 512)
                    pg = spool.tile([128, 512], F32, tag="pg")
                    # copy the contiguous 0..96 partition range (rows between
                    # the 4 preds rows are dead); DMA then strides over them
                    nc.scalar.activation(pg[0:97, :], psP[c][0:97, :], AF.Copy)
                    nc.sync.dma_start(out_d.ap()[e - 3:e + 1, cs],
                                      pg[0:128:32, :])

    nc.compile()
    return nc


def kernel(x, t, mask_token,
           te_w1, te_b1, te_w2, te_b2,
           fproj_w, fproj_b, bproj_w, bproj_b,
           fwz, fbz, fwh, fbh,
           bwz, bbz, bwh, bbh,
           gh_w1, gh_b1, gh_w2, gh_b2):
    inp = dict(te_w1=te_w1, te_b1=te_b1, te_w2=te_w2, te_b2=te_b2,
               fproj_w=fproj_w, fproj_b=fproj_b, bproj_w=bproj_w,
               bproj_b=bproj_b, fwz=fwz, fbz=fbz, fwh=fwh, fbh=fbh,
               bwz=bwz, bbz=bbz, bwh=bwh, bbh=bbh,
               gh_w1=gh_w1, gh_b1=gh_b1, gh_w2=gh_w2, gh_b2=gh_b2)
    wts, b2 = _pack_weights(inp)

    x = np.asarray(x, np.float32)
    t = np.asarray(t, np.float32)
    tok = np.asarray(mask_token, np.float32)
    xT = np.swapaxes(x, 1, 2)                    # (B, 3, L)
    mask = xT[:, 2:3, :]
    x12 = np.where(mask == 0, tok.reshape(1, 2, 1), xT[:, 0:2, :])
    tn = np.swapaxes(t, 1, 2)                    # (B, 1, L)
    inx = np.ascontiguousarray(
        np.concatenate([mask, x12, tn], axis=1), np.float32)  # (B, 4, L)

    if "nc" not in _cache:
        _cache["nc"] = _build_program()
    nc = _cache["nc"]

    in_maps = [
        {"wts": wts, "inx": inx[c * BS:(c + 1) * BS]} for c in range(NCORES)
    ]
    res = run_bass_kernel_spmd(nc, in_maps, core_ids=list(range(NCORES)))
    out = np.concatenate([res.results[c]["out"] for c in range(NCORES)], axis=0)
    return (out + b2).reshape(B, L, 1).astype(np.float32)



# revision 8
# speedup vs baseline: 9.5533x; 9.5533x over previous
"""Bi-directional minGRU kernel for Trainium2 (8 NeuronCores, Bass/Tile).

Strategy (v2)
-------------
Data-parallel over batch: B=256 examples, 32 per core. Per example all tensors
live feature-major [feature->partition, time->free]; linears are TensorE
matmuls with K=features on partitions, and the minGRU recurrence is one
hardware tensor_tensor_scan along time: rows 0-63 forward (normal order),
rows 64-127 backward (reversed order via negative-stride matmul rhs).

v2 changes vs v1:
 - All big matmuls run in float32r (TF32-like 11-bit operand rounding, 4x the
   fp32 PE throughput at free-size >= 256). Numpy modeling of the exact
   rounding chain gives max rel err ~6e-4 vs the 2e-2 gate.
 - The mask broadcast (previously two 512KB DMAs per example, ~300us/core of
   DMA) is now a K=3 zero-padded PE matmul into PSUM: rows 96-98 of the input
   tile hold [m, m_reversed, t]; ones-weights broadcast m into psum rows 0-63
   (normal) and 64-127 (reversed), matching Dt's row layout.
 - The time-encoder first layer is a K=3 matmul on the same rows (w1 on the
   t row, zeros on m rows) + a fused DVE (bias+relu) into the gate input
   tile, freeing ScalarE.
 - Forward/backward gate matmuls run as col-tiled pairs (tile_position (0,0)
   and (0,64)) which the PE can execute concurrently.
 - Head layer 1 accumulates U (te part), W1f (fwd E) and zero-padded W1b
   (reversed bwd E) serially at tile (0,0) into one psum; masked fixup
   E = m*(pre - final) commutes with the head matmul, final's contribution
   is folded into the per-example head bias via a tiny fp32 N=1 matmul.
"""
import os
import sys

for _p in ("/opt/trn_rl_repo", "/root/.axon_site/_ro/trn_rl_repo"):
    if os.path.isdir(_p) and _p not in sys.path:
        sys.path.insert(0, _p)

import numpy as np
from contextlib import ExitStack

import concourse.bacc as bacc
import concourse.tile as tile
import concourse.mybir as mybir
from concourse.bass_utils import run_bass_kernel_spmd

F32 = mybir.dt.float32
F32R = mybir.dt.float32r
AF = mybir.ActivationFunctionType
OP = mybir.AluOpType

B, L, H, TE = 256, 2048, 64, 64
NCORES = 8
BS = B // NCORES          # examples per core
NW = 1108                 # packed weight columns

# weight column layout inside the packed [128, NW] tile.
# fp32r matmuls reject col tile_positions, so every matmul runs at (0,0)
# (or a row offset) with M=128 zero-padded lhsTs where needed.
_C_W1FB = 0               # W1fb^T [128, 128] (rows 0-63 W1f^T, 64-127 W1b^T)
_C_W1B0 = 128             # zero-padded W1b^T [128, 128] (rows 0-63 zero)
_C_ZF = 256               # fwd z lhsT [67, 128], cols 64-127 zero
_C_ZB = 384               # bwd z lhsT [67, 128], cols 0-63 zero
_C_HF = 512
_C_HB = 640
_C_W1T2 = 768             # W1t2^T rows 0-63, [64, 128]
_C_TEM3 = 896             # te lhsT rows 64-69 = [0,0,0,0,0,w1], [6, 64]
_C_M3 = 960               # mask bcast lhsT rows 64-69, [6, 128]
_C_W2T4 = 1088            # 4 blocks [128, 4]: block c has w2 in col c
_C_ZBIAS = 1104           # gate z bias column
_C_HBIAS = 1105           # gate h bias column
_C_HEADB = 1106           # head bias column (gh_b1 + W1t@te_b2)
_C_B1 = 1107              # te bias, rows 0-63

_cache = {}


def _pack_weights(inp):
    """Fuse the linear chains (fp64) and pack every lhsT into one [128, NW]
    fp32 array whose column slices are the matmul stationary operands."""
    g = {k: np.asarray(v, np.float64) for k, v in inp.items()}
    wts = np.zeros((128, NW), np.float64)

    def fuse(proj_w, proj_b, wz, bz, wh, bh):
        P3 = proj_w[:, :3]
        Pte_te2 = proj_w[:, 3:] @ g["te_w2"]
        pbias = proj_w[:, 3:] @ g["te_b2"] + proj_b
        # device x3 rows are ordered [mask, x1, x2]
        reord = np.stack([P3[:, 2], P3[:, 0], P3[:, 1]], axis=1)
        return (
            np.concatenate([wz @ Pte_te2, wz @ reord], axis=1),  # (64, 67)
            wz @ pbias + bz,
            np.concatenate([wh @ Pte_te2, wh @ reord], axis=1),
            wh @ pbias + bh,
        )

    Zf, zbf, Hf, hbf = fuse(g["fproj_w"], g["fproj_b"], g["fwz"], g["fbz"],
                            g["fwh"], g["fbh"])
    Zb, zbb, Hb, hbb = fuse(g["bproj_w"], g["bproj_b"], g["bwz"], g["bbz"],
                            g["bwh"], g["bbh"])
    wts[0:67, _C_ZF:_C_ZF + 64] = Zf.T
    wts[0:67, _C_HF:_C_HF + 64] = Hf.T
    wts[0:67, _C_ZB + 64:_C_ZB + 128] = Zb.T
    wts[0:67, _C_HB + 64:_C_HB + 128] = Hb.T
    wts[0:64, _C_ZBIAS] = zbf
    wts[64:128, _C_ZBIAS] = zbb
    wts[0:64, _C_HBIAS] = hbf
    wts[64:128, _C_HBIAS] = hbb
    # head
    W1f = g["gh_w1"][:, :64]
    W1b = g["gh_w1"][:, 64:128]
    W1t = g["gh_w1"][:, 128:192]
    wts[0:64, _C_W1FB:_C_W1FB + 128] = W1f.T
    wts[64:128, _C_W1FB:_C_W1FB + 128] = W1b.T
    wts[64:128, _C_W1B0:_C_W1B0 + 128] = W1b.T
    wts[0:64, _C_W1T2:_C_W1T2 + 128] = (W1t @ g["te_w2"]).T
    wts[0:128, _C_HEADB] = g["gh_b1"] + W1t @ g["te_b2"]
    for c in range(4):
        wts[0:128, _C_W2T4 + 4 * c + c] = g["gh_w2"][0]
    # te first layer: rows 64-69 = [m, x1, x2, m2, m_rev, t]; w1 on the t row
    wts[69, _C_TEM3:_C_TEM3 + 64] = g["te_w1"][:, 0]
    wts[0:64, _C_B1] = g["te_b1"]
    # mask broadcast: m2 -> psum rows 0-63, m_rev -> psum rows 64-127
    wts[67, _C_M3:_C_M3 + 64] = 1.0
    wts[68, _C_M3 + 64:_C_M3 + 128] = 1.0
    return np.ascontiguousarray(wts, np.float32), np.float32(g["gh_b2"][0])


def _build_program():
    """Build + compile the 8-core SPMD Bass program once."""
    nc = bacc.Bacc("TRN2", num_devices=NCORES, debug=False)
    wts_d = nc.dram_tensor("wts", [128, NW], F32R, kind="ExternalInput")
    inx_d = nc.dram_tensor("inx", [BS, 6, L], F32R, kind="ExternalInput")
    out_d = nc.dram_tensor("out", [BS, 4, 512], F32, kind="ExternalOutput")

    with tile.TileContext(nc) as tc, ExitStack() as ctx:
        wpool = ctx.enter_context(tc.tile_pool(name="w", bufs=1))
        pool = ctx.enter_context(tc.tile_pool(name="p", bufs=2))
        pool3 = ctx.enter_context(tc.tile_pool(name="p3", bufs=3))
        spool = ctx.enter_context(tc.tile_pool(name="s", bufs=2))
        ps_te = ctx.enter_context(tc.tile_pool(name="pte", bufs=2, space="PSUM"))
        ps_g = ctx.enter_context(tc.tile_pool(name="pg", bufs=2, space="PSUM"))
        ps_m = ctx.enter_context(tc.tile_pool(name="pm", bufs=1, space="PSUM"))
        ps_h = ctx.enter_context(tc.tile_pool(name="ph", bufs=2, space="PSUM"))
        ps_o = ctx.enter_context(tc.tile_pool(name="po", bufs=1, space="PSUM"))

        wts = wpool.tile([128, NW], F32R, tag="wts")
        nc.sync.dma_start(wts[:], wts_d.ap()[:])
        wtsF = wts[:].bitcast(F32)
        inx = inx_d.ap()

        for e in range(BS):
            # ---- input staging -------------------------------------------
            xr = pool3.tile([128, L], F32R, tag="xr")
            nc.sync.dma_start(xr[64:70, :], inx[e, :, :])  # m,x1,x2,m,mrev,t
            xrev = xr[0:67, ::-1]

            # ---- time encoder r = relu(w1*t + b1) into xr rows 0-63 ------
            for c in range(4):
                cs = slice(c * 512, (c + 1) * 512)
                pst = ps_te.tile([128, 512], F32, tag="teP")
                nc.tensor.matmul(pst[0:64, :],
                                 wts[64:70, _C_TEM3:_C_TEM3 + 64],
                                 xr[64:70, cs], start=True, stop=True,
                                 tile_position=(64, 0))
                nc.vector.tensor_scalar(xr[0:64, cs], pst[0:64, :],
                                        wts[0:64, _C_B1:_C_B1 + 1].bitcast(F32),
                                        0.0, OP.add, OP.max)

            # ---- gates ----------------------------------------------------
            Z = pool3.tile([128, L], F32, tag="Z")
            TH = pool3.tile([128, L], F32, tag="TH")
            for (dst, cf, cb, bias_c, fn) in (
                    (Z, _C_ZF, _C_ZB, _C_ZBIAS, AF.Sigmoid),
                    (TH, _C_HF, _C_HB, _C_HBIAS, AF.Tanh)):
                for c in range(4):
                    cs = slice(c * 512, (c + 1) * 512)
                    psg = ps_g.tile([128, 512], F32, tag="gP")
                    nc.tensor.matmul(psg[:],
                                     wts[0:67, cf:cf + 128],
                                     xr[0:67, cs], start=True, stop=False,
                                     tile_position=(0, 0))
                    nc.tensor.matmul(psg[:],
                                     wts[0:67, cb:cb + 128],
                                     xrev[:, cs], start=False, stop=True,
                                     tile_position=(0, 0))
                    nc.scalar.activation(dst[:, cs], psg[:], fn,
                                         bias=wtsF[:, bias_c:bias_c + 1])

            # ---- scan inputs: a = 1-z (DVE), b = z*th (Pool) -------------
            A = pool.tile([128, L], F32, tag="A")
            Bt = pool.tile([128, L], F32, tag="Bt")
            nc.gpsimd.tensor_scalar(A[:], Z[:], -1.0, 1.0, OP.mult, OP.add)
            nc.gpsimd.tensor_tensor(Bt[:], Z[:], TH[:], OP.mult)

            # ---- the scan -------------------------------------------------
            Hs = pool3.tile([128, L + 1], F32, tag="Hs")
            nc.vector.memset(Hs[:, 0:1], 0.0)
            nc.vector.tensor_tensor_scan(Hs[:, 1:L + 1], A[:], Bt[:], 0.0,
                                         OP.mult, OP.add)

            # ---- head bias: W1fb @ final + headb (fp32, N=1) -------------
            psv = ps_m.tile([128, 512], F32, tag="mP", name=f"psv{e}")
            nc.tensor.matmul(psv[:, 0:1], wtsF[:, _C_W1FB:_C_W1FB + 128],
                             Hs[:, L - 1:L], start=True, stop=True,
                             tile_position=(0, 0))
            sbb = spool.tile([128, 1], F32, tag="sbb")
            nc.scalar.activation(sbb[:, 0:1], psv[:, 0:1], AF.Identity,
                                 bias=wtsF[:, _C_HEADB:_C_HEADB + 1])

            # ---- Dt = pre - final ----------------------------------------
            Dt = pool.tile([128, L], F32, tag="Dt")
            nc.gpsimd.tensor_scalar(Dt[:], Hs[:, 0:L],
                                    Hs[:, L - 1:L], None, OP.subtract)

            # ---- Et = m * Dt (mask bcast via PE, multiply on Pool) -------
            Et = pool.tile([128, L], F32R, tag="Et")
            for c in range(4):
                cs = slice(c * 512, (c + 1) * 512)
                psm = ps_m.tile([128, 512], F32, tag="mP", name=f"mP{e}_{c}")
                nc.tensor.matmul(psm[:], wts[64:70, _C_M3:_C_M3 + 128],
                                 xr[64:70, cs], start=True, stop=True,
                                 tile_position=(64, 0))
                nc.vector.tensor_tensor(Et[:, cs], Dt[:, cs], psm[:], OP.mult)
            Erev = Et[0:128, ::-1]

            # ---- head layer 1 --------------------------------------------
            hid = pool.tile([128, L], F32R, tag="hid")
            for c in range(4):
                cs = slice(c * 512, (c + 1) * 512)
                psS = ps_h.tile([128, 512], F32, tag="h1P")
                nc.tensor.matmul(psS[:],
                                 wts[0:64, _C_W1T2:_C_W1T2 + 128],
                                 xr[0:64, cs], start=True, stop=False,
                                 tile_position=(0, 0))
                nc.tensor.matmul(psS[:],
                                 wts[0:64, _C_W1FB:_C_W1FB + 128],
                                 Et[0:64, cs], start=False, stop=False,
                                 tile_position=(0, 0))
                nc.tensor.matmul(psS[:],
                                 wts[0:128, _C_W1B0:_C_W1B0 + 128],
                                 Erev[:, cs], start=False, stop=True,
                                 tile_position=(0, 0))
                nc.scalar.activation(hid[:, cs], psS[:], AF.Relu,
                                     bias=sbb[:, 0:1])

            # ---- head layer 2: chunk c -> psum row 32c -------------------
            psP = ps_o.tile([128, 512], F32, tag="oP", name=f"oP{e}")
            for c in range(4):
                cs = slice(c * 512, (c + 1) * 512)
                nc.tensor.matmul(psP[0:4, :],
                                 wts[:, _C_W2T4 + 4 * c:_C_W2T4 + 4 * c + 4],
                                 hid[:, cs], start=(c == 0), stop=(c == 3),
                                 tile_position=(0, 0))
            pg = spool.tile([128, 512], F32, tag="pgo")
            nc.scalar.activation(pg[0:4, :], psP[0:4, :], AF.Copy)
            nc.sync.dma_start(out_d.ap()[e], pg[0:4, :])

    nc.compile()
    return nc


def _prep_inx(x, t, mask_token):
    """Host-side packing of the per-example input rows [m,x1,x2,m,m_rev,t]."""
    x = np.asarray(x, np.float32)
    t = np.asarray(t, np.float32)
    tok = np.asarray(mask_token, np.float32)
    xT = np.swapaxes(x, 1, 2)                    # (B, 3, L)
    mask = xT[:, 2:3, :]
    x12 = np.where(mask == 0, tok.reshape(1, 2, 1), xT[:, 0:2, :])
    tn = np.swapaxes(t, 1, 2)                    # (B, 1, L)
    return np.ascontiguousarray(
        np.concatenate([mask, x12, mask, mask[:, :, ::-1], tn], axis=1),
        np.float32)  # (B, 6, L)


def kernel(x, t, mask_token,
           te_w1, te_b1, te_w2, te_b2,
           fproj_w, fproj_b, bproj_w, bproj_b,
           fwz, fbz, fwh, fbh,
           bwz, bbz, bwh, bbh,
           gh_w1, gh_b1, gh_w2, gh_b2):
    inp = dict(te_w1=te_w1, te_b1=te_b1, te_w2=te_w2, te_b2=te_b2,
               fproj_w=fproj_w, fproj_b=fproj_b, bproj_w=bproj_w,
               bproj_b=bproj_b, fwz=fwz, fbz=fbz, fwh=fwh, fbh=fbh,
               bwz=bwz, bbz=bbz, bwh=bwh, bbh=bbh,
               gh_w1=gh_w1, gh_b1=gh_b1, gh_w2=gh_w2, gh_b2=gh_b2)
    wts, b2 = _pack_weights(inp)
    inx = _prep_inx(x, t, mask_token)

    if "nc" not in _cache:
        _cache["nc"] = _build_program()
    nc = _cache["nc"]

    in_maps = [
        {"wts": wts, "inx": inx[c * BS:(c + 1) * BS]} for c in range(NCORES)
    ]
    res = run_bass_kernel_spmd(nc, in_maps, core_ids=list(range(NCORES)))
    out = np.concatenate([res.results[c]["out"].reshape(BS, L)
                          for c in range(NCORES)], axis=0)
    return (out + b2).reshape(B, L, 1).astype(np.float32)
